# revision 1
# baseline (speedup 1.0000x reference)
"""CRF NLL loss kernel v2: meet-in-the-middle forward/backward split.

Two independent serial chains run concurrently on each core, halving the
1024-step serial latency of v1:
  - forward  chain t = 0..512:  A_t = (E_aug^T @ A_{t-1})[0:64] * X[t]
  - backward chain t = 1023..512:  B_t = E @ (X[t+1]*B_{t+1}) + endexp*ind_t
    implemented as one [65]-row matmul: lhsT_b = [E^T ; endexp_row],
    rhs_t = [X[t+1]*B_{t+1} ; ind_t], where ind_t[b] = (len[b]-1 == t).
    The ind row is refreshed per step by the (otherwise idle) Pool engine.
  - meet at m=512: D_b = sum_j A_512[j,b] * B_512[j,b]  (covers t* >= 512);
    t* = 511 (len=512) covered by d_511 from the forward chain's endexp col.
All in normal space with constant per-step rescale R folded into
X = exp(em - R); host adds back R*(t*+1) after log.
"""

import os
import sys

for _p in ("/opt/trn_rl_repo", "/root/.axon_site/_ro/trn_rl_repo"):
    if os.path.isdir(_p) and _p not in sys.path:
        sys.path.insert(0, _p)

import numpy as np

B, S, T = 512, 1024, 64
NCORES = 8
BL = B // NCORES  # 64
R = float(np.log(64.0) + 0.5)
TB = 16
M = S // 2  # 512, meeting point


def _build_program():
    import concourse.bass as bass
    import concourse.bacc as bacc
    import concourse.mybir as mybir
    from concourse import tile

    f32 = mybir.dt.float32
    bf16 = mybir.dt.bfloat16
    AF = mybir.ActivationFunctionType

    nc = bacc.Bacc(None, target_bir_lowering=False)

    emT = nc.dram_tensor("emT", [T, S * BL], f32, kind="ExternalInput")
    lhsf = nc.dram_tensor("lhsf", [T, T + 1], bf16, kind="ExternalInput")
    lhsb = nc.dram_tensor("lhsb", [T + 1, T], bf16, kind="ExternalInput")
    sx = nc.dram_tensor("sx", [T, 1], f32, kind="ExternalInput")
    indh = nc.dram_tensor("indh", [1, (S - M) * 64], f32, kind="ExternalInput")
    outv = nc.dram_tensor("outv", [1, 2 * BL], f32, kind="ExternalOutput")

    with tile.TileContext(nc) as tc:
        with (
            tc.tile_pool(name="const", bufs=1) as constp,
            tc.tile_pool(name="rawf", bufs=3) as rawfp,
            tc.tile_pool(name="xf", bufs=3) as xfp,
            tc.tile_pool(name="rawb", bufs=3) as rawbp,
            tc.tile_pool(name="xb", bufs=3) as xbp,
            tc.tile_pool(name="astate", bufs=3) as astatep,
            tc.tile_pool(name="brhs", bufs=3) as brhsp,
            tc.tile_pool(name="outp", bufs=1) as outp,
            tc.tile_pool(name="psf", bufs=3, space="PSUM") as psfp,
            tc.tile_pool(name="psb", bufs=3, space="PSUM") as psbp,
            tc.tile_pool(name="pssum", bufs=1, space="PSUM") as pssump,
        ):
            lhsf_t = constp.tile([T, T + 1], bf16)
            nc.sync.dma_start(lhsf_t[:], lhsf[:])
            lhsb_t = constp.tile([T + 1, T], bf16)
            nc.sync.dma_start(lhsb_t[:], lhsb[:])
            sx_t = constp.tile([T, 1], f32)
            nc.sync.dma_start(sx_t[:], sx[:])
            ind_t = constp.tile([1, (S - M) * 64], f32)
            nc.sync.dma_start(ind_t[:], indh[:])
            ones_t = constp.tile([T, 1], bf16)
            nc.gpsimd.memset(ones_t[:], 1.0)
            negr = constp.tile([T, 1], f32)
            nc.gpsimd.memset(negr[:], -R)
            out_t = outp.tile([1, 2 * BL], f32)

            # X block tiles, loaded lazily in chain order
            xf_tiles = {}
            xb_tiles = {}

            def get_xf(blk):
                if blk not in xf_tiles:
                    raw = rawfp.tile([T, TB * BL], f32)
                    nc.sync.dma_start(
                        raw[:], emT[:, blk * TB * BL : (blk + 1) * TB * BL]
                    )
                    xt = xfp.tile([T, TB * BL], f32)
                    nc.scalar.activation(xt[:], raw[:], AF.Exp, bias=negr[:, 0:1])
                    xf_tiles[blk] = xt
                return xf_tiles[blk]

            def get_xb(blk):
                if blk not in xb_tiles:
                    raw = rawbp.tile([T, TB * BL], f32)
                    nc.sync.dma_start(
                        raw[:], emT[:, blk * TB * BL : (blk + 1) * TB * BL]
                    )
                    xt = xbp.tile([T, TB * BL], f32)
                    nc.scalar.activation(xt[:], raw[:], AF.Exp, bias=negr[:, 0:1])
                    xb_tiles[blk] = xt
                return xb_tiles[blk]

            def xslice(xt, t):
                k = t % TB
                return xt[:, k * BL : (k + 1) * BL]

            def ind_slice(t):
                return ind_t[0:1, (t - M) * 64 : (t - M) * 64 + 64]

            # ---- chain initializations ----
            a_prev = astatep.tile([T, BL], bf16)
            nc.vector.tensor_scalar_mul(a_prev[:], xslice(get_xf(0), 0), sx_t[:, 0:1])

            b_rhs = brhsp.tile([T + 1, BL], bf16)
            nc.gpsimd.memset(b_rhs[0:T, :], 0.0)
            nc.gpsimd.tensor_copy(b_rhs[T : T + 1, :], ind_slice(S - 1))

            a_final = None
            b_final_ps = None

            # ---- interleaved chains: fwd t = 1..512, bwd tau = 1023..512 ----
            for s in range(M):
                tf = s + 1  # forward step index
                tb = S - 1 - s  # backward step index

                # forward: P = lhsf^T @ A_{tf-1}; A_tf = P[0:64] * X[tf]
                psf = psfp.tile([T + 1, BL], f32)
                nc.tensor.matmul(psf[:], lhsf_t[:], a_prev[:], start=True, stop=True)
                if tf == M:
                    # record d_511 (endexp row of the final forward matmul)
                    nc.scalar.activation(
                        out_t[0:1, BL : 2 * BL], psf[T : T + 1, :], AF.Copy
                    )
                a_new = astatep.tile([T, BL], bf16)
                nc.vector.tensor_mul(
                    a_new[:], psf[0:T, :], xslice(get_xf(tf // TB), tf)
                )
                a_prev = a_new
                if tf == M:
                    a_final = a_new

                # backward: B_tb = lhsb^T @ rhs_tb;
                # rhs_{tb-1} = [X[tb] * B_tb ; ind_{tb-1}]
                psb = psbp.tile([T, BL], f32)
                nc.tensor.matmul(psb[:], lhsb_t[:], b_rhs[:], start=True, stop=True)
                if tb == M:
                    b_final_ps = psb
                else:
                    nb = brhsp.tile([T + 1, BL], bf16)
                    nc.vector.tensor_mul(
                        nb[0:T, :], psb[:], xslice(get_xb(tb // TB), tb)
                    )
                    nc.gpsimd.tensor_copy(nb[T : T + 1, :], ind_slice(tb - 1))
                    b_rhs = nb

            # ---- meet: D = sum_j A_512[j,b] * B_512[j,b] ----
            mprod = astatep.tile([T, BL], bf16)
            nc.vector.tensor_mul(mprod[:], b_final_ps[:], a_final[:])
            pssum = pssump.tile([1, BL], f32)
            nc.tensor.matmul(pssum[:], ones_t[:], mprod[:], start=True, stop=True)
            nc.scalar.activation(out_t[0:1, 0:BL], pssum[:], AF.Copy)

            nc.sync.dma_start(outv[:], out_t[:])

    nc.compile()
    return nc


_NC_CACHE = None
_RUN_KWARGS: dict = {}
_LAST_RES = None


def kernel(emissions, tags, mask, start_transitions, end_transitions, transitions):
    global _NC_CACHE
    from concourse.bass_utils import run_bass_kernel_spmd

    emissions = np.asarray(emissions, dtype=np.float32)
    tags = np.asarray(tags).astype(np.int64)
    mask = np.asarray(mask).astype(np.int32)
    start = np.asarray(start_transitions, dtype=np.float32)
    end = np.asarray(end_transitions, dtype=np.float32)
    trans = np.asarray(transitions, dtype=np.float32)

    if _NC_CACHE is None:
        _NC_CACHE = _build_program()
    nc = _NC_CACHE

    E64 = np.exp(trans.astype(np.float64))
    endexp = np.exp(end.astype(np.float64))
    import ml_dtypes
    lhsf = np.concatenate([E64, endexp[:, None]], axis=1).astype(ml_dtypes.bfloat16)
    lhsb = np.concatenate([E64.T, endexp[None, :]], axis=0).astype(ml_dtypes.bfloat16)
    sx = np.exp(start.astype(np.float64)).astype(np.float32)[:, None]

    lengths = mask.sum(axis=1).astype(np.int64)
    tstar = lengths - 1  # in [511, 1023]

    in_maps = []
    for c in range(NCORES):
        em_c = emissions[c * BL : (c + 1) * BL]
        emT_c = np.ascontiguousarray(em_c.transpose(2, 1, 0)).reshape(T, S * BL)
        indh = np.zeros((1, (S - M) * 64), np.float32)
        ts_c = tstar[c * BL : (c + 1) * BL]
        for b in range(BL):
            t = int(ts_c[b])
            if t >= M:
                indh[0, (t - M) * 64 + b] = 1.0
        in_maps.append(
            {"emT": emT_c, "lhsf": lhsf, "lhsb": lhsb, "sx": sx, "indh": indh}
        )

    res = run_bass_kernel_spmd(nc, in_maps, list(range(NCORES)), **_RUN_KWARGS)
    globals()["_LAST_RES"] = res

    barange = np.arange(B)

    den = np.empty(B, dtype=np.float64)
    for c in range(NCORES):
        out = res.results[c]["outv"].reshape(-1)  # [2*BL]
        ts_c = tstar[c * BL : (c + 1) * BL]
        comb = out[0:BL].astype(np.float64)
        d511 = out[BL : 2 * BL].astype(np.float64)
        val = np.where(ts_c >= M, comb, d511)
        with np.errstate(divide="ignore", invalid="ignore"):
            den[c * BL : (c + 1) * BL] = np.log(val) + R * (ts_c + 1)

    mk = mask.astype(np.float64)
    score0 = start[tags[:, 0]].astype(np.float64) + emissions[
        barange, 0, tags[:, 0]
    ].astype(np.float64)
    trans_sc = trans[tags[:, :-1], tags[:, 1:]].astype(np.float64)
    emit_sc = np.take_along_axis(emissions[:, 1:, :], tags[:, 1:, None], axis=2)[
        ..., 0
    ].astype(np.float64)
    score = score0 + ((trans_sc + emit_sc) * mk[:, 1:]).sum(axis=1)
    last_tags = tags[barange, lengths - 1]
    num = score + end[last_tags].astype(np.float64)

    ll = num - den
    loss = -(ll.sum() / mk.sum())
    return np.float32(loss)

